# Initial kernel scaffold
#
"""Trainium2 Bass kernel for a 2-layer GNN message-passing encoder.

Math (per layer):  out = segment_mean(x[src] * w, dst) + x @ Wr.T
with w = typew(src,dst) * edge_weight, run twice (Wr1 then Wr2).

Device strategy (8 NeuronCores, SPMD single program):
  - Nodes padded to 50176 = 8 cores * 49 ranks * 128; core c owns the
    contiguous 6272-node range [c*6272, (c+1)*6272), i.e. 49 windows of
    128 nodes. Edges are assigned to the core owning their dst.
  - Per 128-node window, the weighted segment-mean is computed as a sum of
    one-hot matmuls accumulated in PSUM: for each 128-edge tile,
      S[e, n] = (iota[n] == dst_rel[e]) * w'[e]      (one fused DVE op)
      psum[window] += S.T-matmul: matmul(psum, lhsT=S, rhs=x_gathered)
    where w' = typew * edge_weight * 1/max(count(dst),1) is folded on host,
    so PSUM directly accumulates the mean. The root linear x @ Wr.T is one
    more matmul accumulated into the same PSUM bank (lhsT=x.T slice,
    rhs=Wr.T).
  - x[src] rows are fetched with the SWDGE dma_gather custom instruction
    (fp16, 256B rows) straight from DRAM. int16 gather indices can't span
    50176 rows, so each window's edges are split into lo (src < 25088) and
    hi classes; the hi gather uses a base-shifted view of the source.
    Pad slots use idx=0 with w'=0.
  - Between layers, per-core h slices (fp16) are AllGathered to rebuild the
    full gather source for layer 2. h.T (fp32) is kept in SBUF as the
    layer-2 root lhsT.

Host does only index/structure work (sorting, counts, slot packing, dtype
casts/transposes of inputs); all O(E*D) and O(N*D*D) float math runs on
device.
"""

import sys
from contextlib import ExitStack
from dataclasses import dataclass, field

import numpy as np

sys.path.insert(0, "/opt/trn_rl_repo")

import concourse.bacc as bacc  # noqa: E402
import concourse.mybir as mybir  # noqa: E402
import concourse.tile as tile  # noqa: E402
from concourse.bass_utils import run_bass_kernel_spmd  # noqa: E402

D = 128
SAME_W = 0.3
CROSS_W = 1.0


@dataclass
class Cfg:
    n_nodes: int = 50000
    n_cores: int = 8
    ranks_per_core: int = 49
    group: int = 4            # windows per gather batch
    split_rank: int = 196     # lo/hi src split at node 196*128 = 25088
    gather_dtype: str = "float16"
    # SWDGE ring: carveout/64B = descs per engine ring; a gather of T tiles
    # needs T*8+1 descs per engine and must fit well under the ring size.
    dma_scratch: int = 32768
    gather_tiles_max: int = 32
    n_queues: int = 4

    @property
    def npc(self) -> int:           # nodes per core (padded)
        return self.ranks_per_core * 128

    @property
    def npad(self) -> int:
        return self.n_cores * self.npc

    @property
    def split(self) -> int:
        return self.split_rank * 128


@dataclass
class Plan:
    cfg: Cfg
    TL: np.ndarray        # [ranks_per_core] lo-tile capacity per local window
    TH: np.ndarray        # [ranks_per_core] hi-tile capacity per local window
    base_lo: np.ndarray   # [ranks_per_core] tile index of window's lo run
    base_hi: np.ndarray
    groups: list = field(default_factory=list)  # list of lists of local window ids
    # gather instructions: (idx_col_start, slot_tile_start, n_tiles, is_hi)
    ginstrs: list = field(default_factory=list)
    idx_cols: int = 0     # total int16 columns in the gather-index buffer

    @property
    def n_tiles(self) -> int:
        return int(self.TL.sum() + self.TH.sum())


def _make_plan(cfg: Cfg, cnt_lo: np.ndarray, cnt_hi: np.ndarray) -> Plan:
    """cnt_lo/cnt_hi: [n_cores, ranks_per_core] per-window edge counts."""
    RPC = cfg.ranks_per_core
    TL = np.ceil(cnt_lo.max(axis=0) / 128).astype(np.int64)
    TH = np.ceil(cnt_hi.max(axis=0) / 128).astype(np.int64)
    groups = [list(range(q, min(q + cfg.group, RPC))) for q in range(0, RPC, cfg.group)]
    base_lo = np.zeros(RPC, np.int64)
    base_hi = np.zeros(RPC, np.int64)
    t = 0
    runs = []  # (tile_start, n_tiles, is_hi, group_idx) per (group, class) run
    for gi, grp in enumerate(groups):
        lo0 = t
        for wl in grp:
            base_lo[wl] = t
            t += TL[wl]
        runs.append((lo0, t - lo0, False, gi))
        hi0 = t
        for wl in grp:
            base_hi[wl] = t
            t += TH[wl]
        runs.append((hi0, t - hi0, True, gi))
    # chunk runs into gather instructions; each instruction's idx block is
    # 128B-aligned (64 int16 columns) in the index buffer (HW requirement).
    ginstrs = []
    col = 0
    for (t0, n_run, is_hi, gi) in runs:
        done = 0
        while done < n_run:
            n = min(cfg.gather_tiles_max, n_run - done)
            ginstrs.append((col, t0 + done, n, is_hi, gi))
            col += ((n * 8 + 63) // 64) * 64
            done += n
    return Plan(cfg=cfg, TL=TL, TH=TH, base_lo=base_lo, base_hi=base_hi,
                groups=groups, ginstrs=ginstrs, idx_cols=max(col, 64))


def preprocess(x, edge_index, edge_weight, Wr1, Wr2, cell_len, cfg: Cfg):
    """Host-side index/structure prep. Returns (plan, in_maps)."""
    RPC = cfg.ranks_per_core
    src = np.asarray(edge_index[0], dtype=np.int64)
    dst = np.asarray(edge_index[1], dtype=np.int64)
    ew = np.asarray(edge_weight, dtype=np.float32)
    cl = int(np.asarray(cell_len))
    x = np.asarray(x, dtype=np.float32)

    tw = np.where((src > cl) == (dst > cl), SAME_W, CROSS_W).astype(np.float32)
    cnt = np.bincount(dst, minlength=cfg.n_nodes).astype(np.float32)
    inv = (1.0 / np.maximum(cnt, 1.0)).astype(np.float32)
    wfin = tw * ew * inv[dst]

    g = dst >> 7                      # global window id
    klass = (src >= cfg.split).astype(np.int64)   # 0 = lo, 1 = hi
    n_wg = cfg.n_cores * RPC
    gid = g * 2 + klass
    counts = np.bincount(gid, minlength=n_wg * 2)
    cnt_lo = counts[0::2].reshape(cfg.n_cores, RPC)
    cnt_hi = counts[1::2].reshape(cfg.n_cores, RPC)
    plan = _make_plan(cfg, cnt_lo, cnt_hi)

    # slot position of each edge: sorted by (window, class), position in run
    order = np.lexsort((klass, g))
    gid_s = gid[order]
    gid_starts = np.zeros(n_wg * 2 + 1, np.int64)
    np.cumsum(counts, out=gid_starts[1:])
    pos = np.arange(len(src), dtype=np.int64) - gid_starts[gid_s]

    gs = g[order]
    core_e = gs // RPC
    wl_e = gs - core_e * RPC
    kl_e = klass[order]
    tile_base = np.where(kl_e == 0, plan.base_lo[wl_e], plan.base_hi[wl_e])
    n_slots = plan.n_tiles * 128
    slot = core_e * n_slots + tile_base * 128 + pos

    src_s = src[order]
    idx_val = np.where(kl_e == 0, src_s, src_s - cfg.split).astype(np.int16)
    rel_val = (dst[order] - (gs << 7)).astype(np.int64)
    w_val = wfin[order]

    total = cfg.n_cores * n_slots
    idx_slot = np.zeros(total, np.int16)
    idx_slot[slot] = idx_val
    # dense one-hot S, built host-side from indices/weights only:
    # S[core][e, tile, dst_rel] = w'  (fp16); streamed to SBUF per group and
    # used directly as the matmul stationary operand.
    nt = plan.n_tiles
    s_dense = np.zeros((cfg.n_cores, 128, nt, 128), np.float16)
    e_sl = slot % 128
    t_sl = (slot // 128) % nt
    c_sl = slot // (nt * 128)
    s_dense[c_sl, e_sl, t_sl, rel_val] = w_val.astype(np.float16)

    # device-layout constants shared across cores
    xpad16 = np.zeros((cfg.npad, D), np.float16)
    xpad16[: cfg.n_nodes] = x.astype(np.float16)
    xpad32 = np.zeros((cfg.npad, D), np.float32)
    xpad32[: cfg.n_nodes] = x
    w1t = np.ascontiguousarray(np.asarray(Wr1, np.float32).T)
    w2t = np.ascontiguousarray(np.asarray(Wr2, np.float32).T)
    iota16 = np.tile(np.arange(128, dtype=np.float16), (128, 1))
    ident32 = np.eye(128, dtype=np.float32)

    in_maps = []
    for c in range(cfg.n_cores):
        idx_c = idx_slot[c * n_slots : (c + 1) * n_slots]
        g16 = np.zeros((16, plan.idx_cols), np.int16)
        for (c0, t0, n_t, _hi, _gi) in plan.ginstrs:
            g16[:, c0 : c0 + n_t * 8] = idx_c[t0 * 128 : (t0 + n_t) * 128].reshape(
                -1, 16
            ).T
        gidx = np.ascontiguousarray(np.tile(g16, (8, 1)))  # [128, idx_cols]
        xT = np.ascontiguousarray(xpad32[c * cfg.npc : (c + 1) * cfg.npc].T)
        in_maps.append(
            {
                "x16": xpad16,
                "xT32": xT,
                "w1t": w1t,
                "w2t": w2t,
                "gidx": gidx,
                "sden": s_dense[c].reshape(128, plan.n_tiles * 128),
                "ident32": ident32,
            }
        )
    return plan, in_maps


def build_program(plan: Plan, dbg_layers=(0, 1), dbg_gather=True, dbg_tpose=True,
                  dbg_coll=True, dbg_compute=True, dbg_sbuild=True):
    cfg = plan.cfg
    RPC = cfg.ranks_per_core
    dt = mybir.dt
    f32, f16, i16 = dt.float32, dt.float16, dt.int16
    n_tiles = plan.n_tiles
    n_slots = n_tiles * 128

    nc = bacc.Bacc(
        "TRN2",
        target_bir_lowering=False,
        debug=False,
        num_devices=cfg.n_cores,
        dynamic_dma_scratch_size=cfg.dma_scratch,
        num_swdge_queues=cfg.n_queues,
    )
    x16_d = nc.dram_tensor("x16", [cfg.npad, D], f16, kind="ExternalInput")
    xT32_d = nc.dram_tensor("xT32", [D, cfg.npc], f32, kind="ExternalInput")
    w1t_d = nc.dram_tensor("w1t", [D, D], f32, kind="ExternalInput")
    w2t_d = nc.dram_tensor("w2t", [D, D], f32, kind="ExternalInput")
    gidx_d = nc.dram_tensor("gidx", [128, plan.idx_cols], i16, kind="ExternalInput")
    sden_d = nc.dram_tensor("sden", [128, n_tiles * 128], f16, kind="ExternalInput")
    id_d = nc.dram_tensor("ident32", [128, 128], f32, kind="ExternalInput")
    out_d = nc.dram_tensor("out", [cfg.npc, D], f32, kind="ExternalOutput")
    h_slice_d = nc.dram_tensor("h_slice", [cfg.npc, D], f16)
    h_full_d = nc.dram_tensor("h_full", [cfg.npad, D], f16, addr_space="Shared")

    Copy = mybir.ActivationFunctionType.Copy
    is_eq, mult = mybir.AluOpType.is_equal, mybir.AluOpType.mult
    Square = mybir.ActivationFunctionType.Square
    Relu = mybir.ActivationFunctionType.Relu

    with tile.TileContext(nc) as tc, ExitStack() as ctx:
        const = ctx.enter_context(tc.tile_pool(name="const", bufs=1))
        gpool = ctx.enter_context(tc.tile_pool(name="g", bufs=3))
        spool = ctx.enter_context(tc.tile_pool(name="s", bufs=3))
        hpool = ctx.enter_context(tc.tile_pool(name="hw", bufs=4))
        psum_w = ctx.enter_context(tc.tile_pool(name="pw", bufs=5, space="PSUM"))
        psum_t = ctx.enter_context(tc.tile_pool(name="pt", bufs=2, space="PSUM"))

        xT_s = const.tile([D, cfg.npc], f32)
        nc.sync.dma_start(xT_s[:], xT32_d[:, :])
        w1t_s = const.tile([D, D], f32)
        nc.sync.dma_start(w1t_s[:], w1t_d[:, :])
        w2t_s = const.tile([D, D], f32)
        nc.sync.dma_start(w2t_s[:], w2t_d[:, :])
        id_s = const.tile([128, 128], f32)
        nc.sync.dma_start(id_s[:], id_d[:, :])
        gidx_s = const.tile([128, plan.idx_cols], i16)
        nc.sync.dma_start(gidx_s[:], gidx_d[:, :])

        hT_s = const.tile([D, cfg.npc], f32)
        if not dbg_compute:
            nc.vector.memset(hT_s[:], 0.0)

        max_grp_tiles = max(
            int(sum(plan.TL[wl] + plan.TH[wl] for wl in grp)) for grp in plan.groups
        )
        grp_instrs = [[] for _ in plan.groups]
        for inst in plan.ginstrs:
            grp_instrs[inst[4]].append(inst)
        qn = [0]

        for layer in dbg_layers:
            lhsT_root = xT_s if layer == 0 else hT_s
            wt_s = w1t_s if layer == 0 else w2t_s

            for gi, grp in enumerate(plan.groups):
                grp_t0 = int(plan.base_lo[grp[0]])
                n_gt = int(sum(plan.TL[wl] + plan.TH[wl] for wl in grp))
                gw = len(grp)
                if n_gt == 0:
                    gt = None
                    sg = None
                elif not dbg_gather:
                    gt = gpool.tile([128, max_grp_tiles, D], f16, tag="g")
                    nc.vector.memset(gt[:], 0.5)
                    sg = None
                else:
                    gt = gpool.tile([128, max_grp_tiles, D], f16, tag="g")
                    sg = spool.tile([128, max_grp_tiles, D], f16, tag="s")
                    nc.sync.dma_start(
                        sg[:, 0:n_gt, :],
                        sden_d[:, grp_t0 * 128 : (grp_t0 + n_gt) * 128],
                    )
                    if layer == 0:
                        src_lo = x16_d[:, :]
                        src_hi = x16_d[cfg.split :, :]
                    else:
                        src_lo = h_full_d[:, :]
                        src_hi = h_full_d[cfg.split :, :]
                    for (c0, t0, n_t, is_hi, _gi) in grp_instrs[gi]:
                        off = t0 - grp_t0
                        nc.gpsimd.dma_gather(
                            gt[:, off : off + n_t, :],
                            src_hi if is_hi else src_lo,
                            gidx_s[:, c0 : c0 + n_t * 8],
                            n_t * 128,
                            n_t * 128,
                            D,
                            queue_num=qn[0],
                            single_packet=False,
                        )
                        qn[0] = (qn[0] + 1) % cfg.n_queues

                is_last_layer = layer == dbg_layers[-1]
                if is_last_layer:
                    stage = hpool.tile([128, cfg.group, D], f32, tag="ostage")
                else:
                    stage = hpool.tile([128, cfg.group, D], f16, tag="hstage")
                if not dbg_compute:
                    if gt is not None:
                        nc.vector.tensor_copy(stage[:, 0, :], gt[:, 0, :])
                    else:
                        nc.vector.memset(stage[:], 0.0)
                    if not is_last_layer:
                        nc.vector.memset(
                            hT_s[:, grp[0] * 128 : (grp[-1] + 1) * 128], 0.0
                        )
                for wi, wl in enumerate(grp):
                    if not dbg_compute:
                        continue
                    tiles = [int(plan.base_lo[wl]) + i for i in range(int(plan.TL[wl]))]
                    tiles += [int(plan.base_hi[wl]) + i for i in range(int(plan.TH[wl]))]
                    pw = psum_w.tile([128, D], f32, tag="pw")
                    nc.tensor.matmul(
                        pw[:],
                        lhsT_root[:, wl * 128 : (wl + 1) * 128],
                        wt_s[:],
                        start=True,
                        stop=(len(tiles) == 0),
                    )
                    for j, tg in enumerate(tiles):
                        o = tg - grp_t0
                        nc.tensor.matmul(
                            pw[:],
                            sg[:, o, :] if dbg_sbuild else gt[:, o, :],
                            gt[:, o, :],
                            start=False,
                            stop=(j == len(tiles) - 1),
                        )
                    if not is_last_layer:
                        hwin = hpool.tile([128, D], f32, tag="hw")
                        nc.scalar.activation(hwin[:], pw[:], Copy)
                        if dbg_tpose:
                            pt = psum_t.tile([128, D], f32, tag="pt")
                            nc.tensor.transpose(pt[:], hwin[:], id_s[:])
                            nc.scalar.activation(
                                hT_s[:, wl * 128 : (wl + 1) * 128], pt[:], Copy
                            )
                        nc.vector.tensor_copy(stage[:, wi, :], hwin[:])
                    else:
                        nc.scalar.activation(stage[:, wi, :], pw[:], Copy)
                # flush this group's staging to DRAM
                r0, r1 = grp[0] * 128, (grp[-1] + 1) * 128
                if is_last_layer:
                    nc.sync.dma_start(
                        out_d[r0:r1, :].rearrange("(w p) d -> p w d", p=128),
                        stage[:, 0:gw, :],
                    )
                else:
                    nc.sync.dma_start(
                        h_slice_d[r0:r1, :].rearrange("(w p) d -> p w d", p=128),
                        stage[:, 0:gw, :],
                    )

            if layer == 0 and len(dbg_layers) > 1:
                if dbg_coll:
                    nc.gpsimd.collective_compute(
                        "AllGather",
                        mybir.AluOpType.bypass,
                        replica_groups=[list(range(cfg.n_cores))],
                        ins=[h_slice_d[:, :]],
                        outs=[h_full_d[:, :]],
                    )
                else:
                    nc.sync.dma_start(h_full_d[0 : cfg.npc, :], h_slice_d[:, :])

    nc.compile()
    return nc


_CACHE: dict = {}


def _get_program(plan: Plan):
    key = (
        plan.cfg.n_nodes,
        plan.cfg.n_cores,
        plan.cfg.ranks_per_core,
        plan.cfg.group,
        tuple(plan.TL.tolist()),
        tuple(plan.TH.tolist()),
    )
    if key not in _CACHE:
        _CACHE[key] = build_program(plan)
    return _CACHE[key]


def kernel(x, edge_index, edge_weight, Wr1, Wr2, cell_len):
    cfg = Cfg()
    assert x.shape == (cfg.n_nodes, D)
    plan, in_maps = preprocess(x, edge_index, edge_weight, Wr1, Wr2, cell_len, cfg)
    nc = _get_program(plan)
    res = run_bass_kernel_spmd(nc, in_maps, list(range(cfg.n_cores)))
    out = np.concatenate([res.results[c]["out"] for c in range(cfg.n_cores)], axis=0)
    return np.ascontiguousarray(out[: cfg.n_nodes]).astype(np.float32)



# revision 1
# speedup vs baseline: 1.2995x; 1.2995x over previous
"""Trainium2 Bass kernel for a 2-layer GNN message-passing encoder.

Math (per layer):  out = segment_mean(x[src] * w, dst) + x @ Wr.T
with w = typew(src,dst) * edge_weight, run twice (Wr1 then Wr2).

Device strategy (8 NeuronCores, SPMD single program):
  - Nodes padded to 50176 = 8 cores * 49 ranks * 128; core c owns the
    contiguous 6272-node range [c*6272, (c+1)*6272), i.e. 49 windows of
    128 nodes. Edges are assigned to the core owning their dst.
  - Per 128-node window, the weighted segment-mean is computed as a sum of
    one-hot matmuls accumulated in PSUM: for each 128-edge tile,
      S[e, n] = (iota[n] == dst_rel[e]) * w'[e]      (one fused DVE op)
      psum[window] += S.T-matmul: matmul(psum, lhsT=S, rhs=x_gathered)
    where w' = typew * edge_weight * 1/max(count(dst),1) is folded on host,
    so PSUM directly accumulates the mean. The root linear x @ Wr.T is one
    more matmul accumulated into the same PSUM bank (lhsT=x.T slice,
    rhs=Wr.T).
  - x[src] rows are fetched with the SWDGE dma_gather custom instruction
    (fp16, 256B rows) straight from DRAM. int16 gather indices can't span
    50176 rows, so each window's edges are split into lo (src < 25088) and
    hi classes; the hi gather uses a base-shifted view of the source.
    Pad slots use idx=0 with w'=0.
  - Between layers, per-core h slices (fp16) are AllGathered to rebuild the
    full gather source for layer 2. h.T (fp32) is kept in SBUF as the
    layer-2 root lhsT.

Host does only index/structure work (sorting, counts, slot packing, dtype
casts/transposes of inputs); all O(E*D) and O(N*D*D) float math runs on
device.
"""

import sys
from contextlib import ExitStack
from dataclasses import dataclass, field

import numpy as np

sys.path.insert(0, "/opt/trn_rl_repo")

import concourse.bacc as bacc  # noqa: E402
import concourse.mybir as mybir  # noqa: E402
import concourse.tile as tile  # noqa: E402
from concourse.bass_utils import run_bass_kernel_spmd  # noqa: E402

D = 128
SAME_W = 0.3
CROSS_W = 1.0


@dataclass
class Cfg:
    n_nodes: int = 50000
    n_cores: int = 8
    ranks_per_core: int = 49
    group: int = 4            # windows per gather batch
    split_rank: int = 196     # lo/hi src split at node 196*128 = 25088
    gather_dtype: str = "float16"
    # SWDGE ring: carveout/64B = descs per engine ring; a gather of T tiles
    # needs T*8+1 descs per engine and must fit well under the ring size.
    dma_scratch: int = 32768
    gather_tiles_max: int = 32
    n_queues: int = 4

    @property
    def npc(self) -> int:           # nodes per core (padded)
        return self.ranks_per_core * 128

    @property
    def npad(self) -> int:
        return self.n_cores * self.npc

    @property
    def split(self) -> int:
        return self.split_rank * 128


@dataclass
class Plan:
    cfg: Cfg
    TL: np.ndarray        # [ranks_per_core] lo-tile capacity per local window
    TH: np.ndarray        # [ranks_per_core] hi-tile capacity per local window
    base_lo: np.ndarray   # [ranks_per_core] tile index of window's lo run
    base_hi: np.ndarray
    groups: list = field(default_factory=list)  # list of lists of local window ids
    # gather instructions: (idx_col_start, slot_tile_start, n_tiles, is_hi)
    ginstrs: list = field(default_factory=list)
    idx_cols: int = 0     # total int16 columns in the gather-index buffer

    @property
    def n_tiles(self) -> int:
        return int(self.TL.sum() + self.TH.sum())


def _make_plan(cfg: Cfg, cnt_lo: np.ndarray, cnt_hi: np.ndarray) -> Plan:
    """cnt_lo/cnt_hi: [n_cores, ranks_per_core] per-window edge counts."""
    RPC = cfg.ranks_per_core
    TL = np.ceil(cnt_lo.max(axis=0) / 128).astype(np.int64)
    TH = np.ceil(cnt_hi.max(axis=0) / 128).astype(np.int64)
    groups = [list(range(q, min(q + cfg.group, RPC))) for q in range(0, RPC, cfg.group)]
    base_lo = np.zeros(RPC, np.int64)
    base_hi = np.zeros(RPC, np.int64)
    t = 0
    runs = []  # (tile_start, n_tiles, is_hi, group_idx) per (group, class) run
    for gi, grp in enumerate(groups):
        lo0 = t
        for wl in grp:
            base_lo[wl] = t
            t += TL[wl]
        runs.append((lo0, t - lo0, False, gi))
        hi0 = t
        for wl in grp:
            base_hi[wl] = t
            t += TH[wl]
        runs.append((hi0, t - hi0, True, gi))
    # chunk runs into gather instructions; each instruction's idx block is
    # 128B-aligned (64 int16 columns) in the index buffer (HW requirement).
    ginstrs = []
    col = 0
    for (t0, n_run, is_hi, gi) in runs:
        done = 0
        while done < n_run:
            n = min(cfg.gather_tiles_max, n_run - done)
            ginstrs.append((col, t0 + done, n, is_hi, gi))
            col += ((n * 8 + 63) // 64) * 64
            done += n
    return Plan(cfg=cfg, TL=TL, TH=TH, base_lo=base_lo, base_hi=base_hi,
                groups=groups, ginstrs=ginstrs, idx_cols=max(col, 64))


def preprocess(x, edge_index, edge_weight, Wr1, Wr2, cell_len, cfg: Cfg):
    """Host-side index/structure prep. Returns (plan, in_maps)."""
    RPC = cfg.ranks_per_core
    src = np.asarray(edge_index[0], dtype=np.int64)
    dst = np.asarray(edge_index[1], dtype=np.int64)
    ew = np.asarray(edge_weight, dtype=np.float32)
    cl = int(np.asarray(cell_len))
    x = np.asarray(x, dtype=np.float32)

    tw = np.where((src > cl) == (dst > cl), SAME_W, CROSS_W).astype(np.float32)
    cnt = np.bincount(dst, minlength=cfg.n_nodes).astype(np.float32)
    inv = (1.0 / np.maximum(cnt, 1.0)).astype(np.float32)
    wfin = tw * ew * inv[dst]

    g = dst >> 7                      # global window id
    klass = (src >= cfg.split).astype(np.int64)   # 0 = lo, 1 = hi
    n_wg = cfg.n_cores * RPC
    gid = g * 2 + klass
    counts = np.bincount(gid, minlength=n_wg * 2)
    cnt_lo = counts[0::2].reshape(cfg.n_cores, RPC)
    cnt_hi = counts[1::2].reshape(cfg.n_cores, RPC)
    plan = _make_plan(cfg, cnt_lo, cnt_hi)

    # slot position of each edge: sorted by (window, class), position in run
    order = np.lexsort((klass, g))
    gid_s = gid[order]
    gid_starts = np.zeros(n_wg * 2 + 1, np.int64)
    np.cumsum(counts, out=gid_starts[1:])
    pos = np.arange(len(src), dtype=np.int64) - gid_starts[gid_s]

    gs = g[order]
    core_e = gs // RPC
    wl_e = gs - core_e * RPC
    kl_e = klass[order]
    tile_base = np.where(kl_e == 0, plan.base_lo[wl_e], plan.base_hi[wl_e])
    n_slots = plan.n_tiles * 128
    slot = core_e * n_slots + tile_base * 128 + pos

    src_s = src[order]
    idx_val = np.where(kl_e == 0, src_s, src_s - cfg.split).astype(np.int16)
    rel_val = (dst[order] - (gs << 7)).astype(np.int64)
    w_val = wfin[order]

    total = cfg.n_cores * n_slots
    idx_slot = np.zeros(total, np.int16)
    idx_slot[slot] = idx_val
    # dense one-hot S, built host-side from indices/weights only:
    # S[core][e, tile, dst_rel] = w'  (fp16); streamed to SBUF per group and
    # used directly as the matmul stationary operand.
    nt = plan.n_tiles
    s_dense = np.zeros((cfg.n_cores, 128, nt, 128), np.float16)
    e_sl = slot % 128
    t_sl = (slot // 128) % nt
    c_sl = slot // (nt * 128)
    s_dense[c_sl, e_sl, t_sl, rel_val] = w_val.astype(np.float16)

    # device-layout constants shared across cores
    xpad16 = np.zeros((cfg.npad, D), np.float16)
    xpad16[: cfg.n_nodes] = x.astype(np.float16)
    xpad32 = np.zeros((cfg.npad, D), np.float32)
    xpad32[: cfg.n_nodes] = x
    w1t = np.ascontiguousarray(np.asarray(Wr1, np.float32).T)
    w2t = np.ascontiguousarray(np.asarray(Wr2, np.float32).T)
    iota16 = np.tile(np.arange(128, dtype=np.float16), (128, 1))
    ident32 = np.eye(128, dtype=np.float32)

    in_maps = []
    for c in range(cfg.n_cores):
        idx_c = idx_slot[c * n_slots : (c + 1) * n_slots]
        g16 = np.zeros((16, plan.idx_cols), np.int16)
        for (c0, t0, n_t, _hi, _gi) in plan.ginstrs:
            g16[:, c0 : c0 + n_t * 8] = idx_c[t0 * 128 : (t0 + n_t) * 128].reshape(
                -1, 16
            ).T
        gidx = np.ascontiguousarray(np.tile(g16, (8, 1)))  # [128, idx_cols]
        xT = np.ascontiguousarray(xpad32[c * cfg.npc : (c + 1) * cfg.npc].T)
        in_maps.append(
            {
                "x16": xpad16,
                "xT32": xT,
                "w1t": w1t,
                "w2t": w2t,
                "gidx": gidx,
                "sden": s_dense[c].reshape(128, plan.n_tiles * 128),
                "ident32": ident32,
            }
        )
    return plan, in_maps


def build_program(plan: Plan, dbg_layers=(0, 1), dbg_gather=True, dbg_tpose=True,
                  dbg_coll=True, dbg_compute=True, dbg_sbuild=True):
    cfg = plan.cfg
    RPC = cfg.ranks_per_core
    dt = mybir.dt
    f32, f16, i16 = dt.float32, dt.float16, dt.int16
    n_tiles = plan.n_tiles
    n_slots = n_tiles * 128

    nc = bacc.Bacc(
        "TRN2",
        target_bir_lowering=False,
        debug=False,
        num_devices=cfg.n_cores,
        dynamic_dma_scratch_size=cfg.dma_scratch,
        num_swdge_queues=cfg.n_queues,
    )
    x16_d = nc.dram_tensor("x16", [cfg.npad, D], f16, kind="ExternalInput")
    xT32_d = nc.dram_tensor("xT32", [D, cfg.npc], f32, kind="ExternalInput")
    w1t_d = nc.dram_tensor("w1t", [D, D], f32, kind="ExternalInput")
    w2t_d = nc.dram_tensor("w2t", [D, D], f32, kind="ExternalInput")
    gidx_d = nc.dram_tensor("gidx", [128, plan.idx_cols], i16, kind="ExternalInput")
    sden_d = nc.dram_tensor("sden", [128, n_tiles * 128], f16, kind="ExternalInput")
    id_d = nc.dram_tensor("ident32", [128, 128], f32, kind="ExternalInput")
    out_d = nc.dram_tensor("out", [cfg.npc, D], f32, kind="ExternalOutput")
    h_slice_d = nc.dram_tensor("h_slice", [cfg.npc, D], f16)
    h_full_d = nc.dram_tensor("h_full", [cfg.npad, D], f16, addr_space="Shared")

    Copy = mybir.ActivationFunctionType.Copy
    is_eq, mult = mybir.AluOpType.is_equal, mybir.AluOpType.mult
    Square = mybir.ActivationFunctionType.Square
    Relu = mybir.ActivationFunctionType.Relu

    with tile.TileContext(nc) as tc, ExitStack() as ctx:
        const = ctx.enter_context(tc.tile_pool(name="const", bufs=1))
        gpool = ctx.enter_context(tc.tile_pool(name="g", bufs=3))
        spool = ctx.enter_context(tc.tile_pool(name="s", bufs=3))
        hpool = ctx.enter_context(tc.tile_pool(name="hw", bufs=4))
        psum_w = ctx.enter_context(tc.tile_pool(name="pw", bufs=5, space="PSUM"))
        psum_t = ctx.enter_context(tc.tile_pool(name="pt", bufs=2, space="PSUM"))

        xT_s = const.tile([D, cfg.npc], f32)
        nc.sync.dma_start(xT_s[:], xT32_d[:, :])
        w1t_s = const.tile([D, D], f32)
        nc.sync.dma_start(w1t_s[:], w1t_d[:, :])
        w2t_s = const.tile([D, D], f32)
        nc.sync.dma_start(w2t_s[:], w2t_d[:, :])
        id_s = const.tile([128, 128], f32)
        nc.sync.dma_start(id_s[:], id_d[:, :])
        gidx_s = const.tile([128, plan.idx_cols], i16)
        nc.sync.dma_start(gidx_s[:], gidx_d[:, :])

        hT_s = const.tile([D, cfg.npc], f32)
        if not dbg_compute:
            nc.vector.memset(hT_s[:], 0.0)

        max_grp_tiles = max(
            int(sum(plan.TL[wl] + plan.TH[wl] for wl in grp)) for grp in plan.groups
        )
        grp_instrs = [[] for _ in plan.groups]
        for inst in plan.ginstrs:
            grp_instrs[inst[4]].append(inst)
        qn = [0]

        for layer in dbg_layers:
            lhsT_root = xT_s if layer == 0 else hT_s
            wt_s = w1t_s if layer == 0 else w2t_s

            for gi, grp in enumerate(plan.groups):
                grp_t0 = int(plan.base_lo[grp[0]])
                n_gt = int(sum(plan.TL[wl] + plan.TH[wl] for wl in grp))
                gw = len(grp)
                if n_gt == 0:
                    gt = None
                    sg = None
                elif not dbg_gather:
                    gt = gpool.tile([128, max_grp_tiles, D], f16, tag="g")
                    nc.vector.memset(gt[:], 0.5)
                    sg = None
                else:
                    gt = gpool.tile([128, max_grp_tiles, D], f16, tag="g")
                    sg = spool.tile([128, max_grp_tiles, D], f16, tag="s")
                    nc.sync.dma_start(
                        sg[:, 0:n_gt, :],
                        sden_d[:, grp_t0 * 128 : (grp_t0 + n_gt) * 128],
                    )
                    if layer == 0:
                        src_lo = x16_d[:, :]
                        src_hi = x16_d[cfg.split :, :]
                    else:
                        src_lo = h_full_d[:, :]
                        src_hi = h_full_d[cfg.split :, :]
                    for (c0, t0, n_t, is_hi, _gi) in grp_instrs[gi]:
                        off = t0 - grp_t0
                        nc.gpsimd.dma_gather(
                            gt[:, off : off + n_t, :],
                            src_hi if is_hi else src_lo,
                            gidx_s[:, c0 : c0 + n_t * 8],
                            n_t * 128,
                            n_t * 128,
                            D,
                            queue_num=qn[0],
                            single_packet=False,
                        )
                        qn[0] = (qn[0] + 1) % cfg.n_queues

                is_last_layer = layer == dbg_layers[-1]
                if is_last_layer:
                    stage = hpool.tile([128, cfg.group, D], f32, tag="ostage")
                else:
                    stage = hpool.tile([128, cfg.group, D], f16, tag="hstage")
                if not dbg_compute:
                    if gt is not None:
                        nc.vector.tensor_copy(stage[:, 0, :], gt[:, 0, :])
                    else:
                        nc.vector.memset(stage[:], 0.0)
                    if not is_last_layer:
                        nc.vector.memset(
                            hT_s[:, grp[0] * 128 : (grp[-1] + 1) * 128], 0.0
                        )
                for wi, wl in enumerate(grp):
                    if not dbg_compute:
                        continue
                    tiles = [int(plan.base_lo[wl]) + i for i in range(int(plan.TL[wl]))]
                    tiles += [int(plan.base_hi[wl]) + i for i in range(int(plan.TH[wl]))]
                    pw = psum_w.tile([128, D], f32, tag="pw")
                    nc.tensor.matmul(
                        pw[:],
                        lhsT_root[:, wl * 128 : (wl + 1) * 128],
                        wt_s[:],
                        start=True,
                        stop=(len(tiles) == 0),
                    )
                    for j, tg in enumerate(tiles):
                        o = tg - grp_t0
                        nc.tensor.matmul(
                            pw[:],
                            sg[:, o, :] if dbg_sbuild else gt[:, o, :],
                            gt[:, o, :],
                            start=False,
                            stop=(j == len(tiles) - 1),
                        )
                    if not is_last_layer:
                        hwin = hpool.tile([128, D], f32, tag="hw")
                        nc.scalar.activation(hwin[:], pw[:], Copy)
                        if dbg_tpose:
                            pt = psum_t.tile([128, D], f32, tag="pt")
                            nc.tensor.transpose(pt[:], hwin[:], id_s[:])
                            nc.scalar.activation(
                                hT_s[:, wl * 128 : (wl + 1) * 128], pt[:], Copy
                            )
                        nc.vector.tensor_copy(stage[:, wi, :], hwin[:])
                    else:
                        nc.scalar.activation(stage[:, wi, :], pw[:], Copy)
                # flush this group's staging to DRAM
                r0, r1 = grp[0] * 128, (grp[-1] + 1) * 128
                if is_last_layer:
                    nc.sync.dma_start(
                        out_d[r0:r1, :].rearrange("(w p) d -> p w d", p=128),
                        stage[:, 0:gw, :],
                    )
                else:
                    nc.sync.dma_start(
                        h_slice_d[r0:r1, :].rearrange("(w p) d -> p w d", p=128),
                        stage[:, 0:gw, :],
                    )

            if layer == 0 and len(dbg_layers) > 1:
                if dbg_coll:
                    nc.gpsimd.collective_compute(
                        "AllGather",
                        mybir.AluOpType.bypass,
                        replica_groups=[list(range(cfg.n_cores))],
                        ins=[h_slice_d[:, :]],
                        outs=[h_full_d[:, :]],
                    )
                else:
                    nc.sync.dma_start(h_full_d[0 : cfg.npc, :], h_slice_d[:, :])

    nc.compile()
    return nc


_CACHE: dict = {}


def _get_program(plan: Plan):
    key = (
        plan.cfg.n_nodes,
        plan.cfg.n_cores,
        plan.cfg.ranks_per_core,
        plan.cfg.group,
        tuple(plan.TL.tolist()),
        tuple(plan.TH.tolist()),
    )
    if key not in _CACHE:
        _CACHE[key] = build_program(plan)
    return _CACHE[key]


def kernel(x, edge_index, edge_weight, Wr1, Wr2, cell_len):
    cfg = Cfg()
    assert x.shape == (cfg.n_nodes, D)
    plan, in_maps = preprocess(x, edge_index, edge_weight, Wr1, Wr2, cell_len, cfg)
    nc = _get_program(plan)
    res = run_bass_kernel_spmd(nc, in_maps, list(range(cfg.n_cores)))
    out = np.concatenate([res.results[c]["out"] for c in range(cfg.n_cores)], axis=0)
    return np.ascontiguousarray(out[: cfg.n_nodes]).astype(np.float32)

